# revision 11
# baseline (speedup 1.0000x reference)
"""Distributed attention kernel for Trainium2 (8 NeuronCores).

Computes, matching the reference:
    Q = x @ Wq.T + bq; K = x @ Wk.T + bk; V = x @ Wv.T + bv        [B,S,A]
    dots = Q @ K.T (per batch)                                      [B,S,S]
    attn = softmax(dots, axis=0)            # softmax over the BATCH dim
    out  = (attn @ V) @ Wp.T + bp                                   [B,S,F]

Sharding: sequence (rows of Q) is split across the 8 cores (256 rows each,
all 4 batches per core, since the batch-softmax couples batches). Each core
also computes K^T and V for its 256 sequence rows and all-gathers them.

Schedule: projections run K -> V -> Q so each gather's ~30-50us mesh latency
hides under the remaining projections.  Three collectives only (each op has
~15us fixed mesh overhead + transfer): K batches 0-1, K batches 2-3, V.
All DRAM<->SBUF transfers use p-major layouts (partition index outermost on
the host side) so each DMA is 128 descriptors of 2-16KB contiguous runs.

Matmul dtypes: x and Wq/Wk/Wv in fp16 (halves HBM load traffic vs fp32 while
keeping dots errors ~1e-2 absolute), dots in fp16, attn@V / output
projection in bf16.  All accumulation is fp32 in PSUM.  exp() is taken with
a constant -30 shift (softmax is shift-invariant; keeps e^x in fp32 range
for dots in [-82,90]).  Output is written bf16 and cast to f32 on host.
"""

import numpy as np
import ml_dtypes

import concourse.bass as bass
import concourse.tile as tile
from concourse import bacc, mybir
from concourse.bass_utils import run_bass_kernel_spmd

AF = mybir.ActivationFunctionType
F32 = mybir.dt.float32
F16 = mybir.dt.float16
BF16 = mybir.dt.bfloat16

B, S, F, A = 4, 2048, 1024, 1024
NCORES = 8
SQ = S // NCORES          # 256 q rows per core
NFT = F // 128            # 8 f-tiles
NAT = A // 128            # 8 a-tiles
NKT = S // 128            # 16 global k-tiles
AH = A // 2               # 512, a-halves for V
RG = [list(range(NCORES))]


def build():
    nc = bacc.Bacc("TRN2", target_bir_lowering=False, debug=False)

    # p-major layouts: leading dim is the SBUF partition index.
    xt_ext = nc.declare_dram_parameter("xt", [B, 128, NFT * SQ], F16, isOutput=False)
    wqt_ext = nc.declare_dram_parameter("wqt", [128, NFT * A], F16, isOutput=False)
    wkt_ext = nc.declare_dram_parameter("wkt", [128, NFT * A], F16, isOutput=False)
    wvt_ext = nc.declare_dram_parameter("wvt", [128, NFT * A], F16, isOutput=False)
    wpt_ext = nc.declare_dram_parameter("wpt", [128, NAT * F], BF16, isOutput=False)
    bq_ext = nc.declare_dram_parameter("bq", [128, NAT], F32, isOutput=False)
    bk_ext = nc.declare_dram_parameter("bk", [128, NAT], F32, isOutput=False)
    bv_ext = nc.declare_dram_parameter("bv", [1, A], F16, isOutput=False)
    bp_ext = nc.declare_dram_parameter("bp", [1, F], BF16, isOutput=False)
    ones_h_ext = nc.declare_dram_parameter("ones_h", [1, 128], F16, isOutput=False)
    ones_b_ext = nc.declare_dram_parameter("ones_b", [1, 128], BF16, isOutput=False)
    out_ext = nc.declare_dram_parameter("out", [B, SQ, F], BF16, isOutput=True)

    with tile.TileContext(nc) as tc:
        with (
            tc.tile_pool(name="dram", bufs=1, space="DRAM") as dram,
            tc.tile_pool(name="live", bufs=1) as live,
        ):
            # ------------- bounce + gather DRAM buffers (p-major) -------------
            # kb[b][p][at][k]; gathered kgX[c][bl][p][at][k]
            kb = dram.tile([B, 128, NAT, SQ], F16, tag="kb", name="kb")
            kg = dram.tile([NCORES, B, 128, NAT, SQ], F16, addr_space="Shared",
                           tag="kg", name="kg")
            # vb[h][p][st][b][a-half]; gathered vg[h][c][p][st][b][a-half]
            vb = [dram.tile([128, 2, B, AH], BF16, tag=f"vb{h}", name=f"vb{h}")
                  for h in range(2)]
            vg = [dram.tile([NCORES, 128, 2, B, AH], BF16, addr_space="Shared",
                            tag=f"vg{h}", name=f"vg{h}") for h in range(2)]

            # ---------------- whole-kernel SBUF residents ----------------
            bq_sb = live.tile([128, NAT], F32)
            bk_sb = live.tile([128, NAT], F32)
            bv_sb = live.tile([1, A], F16)
            bp_sb = live.tile([1, F], BF16)
            ones_h = live.tile([1, 128], F16)
            ones_b = live.tile([1, 128], BF16)
            negc = live.tile([128, 1], F32)
            nc.gpsimd.memset(negc[:], -30.0)

            qt_sb = [live.tile([128, NAT * SQ], F16, tag=f"qt{b}", name=f"qt{b}")
                     for b in range(B)]
            W_ = [[live.tile([128, SQ], BF16, tag=f"W{b}_{kt}", name=f"W{b}_{kt}")
                   for kt in range(NKT)] for b in range(B)]
            wp_sb = live.tile([128, NAT * F], BF16, tag="wp", name="wp_sb")

            # ============ phase A: projections (K, V, Q) ============
            psA = tc.tile_pool(name="psA", bufs=1, space="PSUM")
            psAp = psA.__enter__()
            with (
                tc.tile_pool(name="phA", bufs=1) as phA,
                tc.tile_pool(name="kout", bufs=12) as kout,
                tc.tile_pool(name="vout", bufs=4) as vout,
            ):
                # wk on the gpsimd queue, xt + the rest on sync: both start
                # at t=0 so the K projection can begin early.
                wk_sb = phA.tile([128, NFT * A], F16, tag="wk", name="wk_sb")
                nc.gpsimd.dma_start(wk_sb[:], wkt_ext[:])
                nc.scalar.dma_start(bq_sb[:], bq_ext[:])
                nc.scalar.dma_start(bk_sb[:], bk_ext[:])
                nc.scalar.dma_start(bv_sb[:], bv_ext[:])
                nc.scalar.dma_start(bp_sb[:], bp_ext[:])
                nc.scalar.dma_start(ones_h[:], ones_h_ext[:])
                nc.scalar.dma_start(ones_b[:], ones_b_ext[:])
                xt_sb = []
                for b in range(B):
                    t = phA.tile([128, NFT * SQ], F16, tag=f"xt{b}", name=f"xt{b}")
                    eng = nc.sync if b < 2 else nc.scalar
                    eng.dma_start(t[:], xt_ext[b])
                    xt_sb.append(t)
                wv_sb = phA.tile([128, NFT * A], F16, tag="wv", name="wv_sb")
                nc.scalar.dma_start(wv_sb[:], wvt_ext[:])
                wq_sb = phA.tile([128, NFT * A], F16, tag="wq", name="wq_sb")
                nc.scalar.dma_start(wq_sb[:], wqt_ext[:])
                nc.scalar.dma_start(wp_sb[:], wpt_ext[:])

                # ---- K^T projection; gather b01 as soon as b0,b1 are done,
                # gather b23 after b2,b3.
                for b in range(B):
                    for at in range(NAT):
                        ps = psAp.tile([128, SQ], F32, tag="pk", bufs=6)
                        for ft in range(NFT):
                            nc.tensor.matmul(
                                ps[:],
                                wk_sb[:, (ft * NAT + at) * 128 : (ft * NAT + at) * 128 + 128],
                                xt_sb[b][:, ft * SQ : (ft + 1) * SQ],
                                start=(ft == 0),
                                stop=(ft == NFT - 1),
                            )
                        kt_t = kout.tile([128, SQ], F16, tag="ko")
                        nc.scalar.activation(
                            kt_t[:], ps[:], AF.Identity, bias=bk_sb[:, at : at + 1]
                        )
                        nc.sync.dma_start(kb[b, :, at, :], kt_t[:])
                    if b == B - 1:
                        nc.gpsimd.collective_compute(
                            "AllGather", mybir.AluOpType.bypass, replica_groups=RG,
                            ins=[kb[:].opt()], outs=[kg[:].opt()],
                        )

                # ---- V projection (a-half outer); per-half all-gather so the
                # first half's gather starts at the V-proj midpoint.
                for h in range(2):
                    for b in range(B):
                        for st in range(2):
                            ps = psAp.tile([128, AH], F32, tag="pv", bufs=2)
                            for ft in range(NFT):
                                nc.tensor.matmul(
                                    ps[:],
                                    xt_sb[b][:, ft * SQ + st * 128 : ft * SQ + st * 128 + 128],
                                    wv_sb[:, ft * A + h * AH : ft * A + h * AH + AH],
                                    start=(ft == 0),
                                    stop=False,
                                )
                            nc.tensor.matmul(
                                ps[:], ones_h[:], bv_sb[:, h * AH : h * AH + AH],
                                start=False, stop=True,
                            )
                            v_t = vout.tile([128, AH], BF16, tag="vo")
                            nc.scalar.activation(v_t[:], ps[:], AF.Copy)
                            nc.scalar.dma_start(vb[h][:, st, b, :], v_t[:])
                    nc.gpsimd.collective_compute(
                        "AllGather", mybir.AluOpType.bypass, replica_groups=RG,
                        ins=[vb[h][:].opt()], outs=[vg[h][:].opt()],
                    )

                # ---- Q^T projection
                for b in range(B):
                    for at in range(NAT):
                        ps = psAp.tile([128, SQ], F32, tag="pk", bufs=6)
                        for ft in range(NFT):
                            nc.tensor.matmul(
                                ps[:],
                                wq_sb[:, (ft * NAT + at) * 128 : (ft * NAT + at) * 128 + 128],
                                xt_sb[b][:, ft * SQ : (ft + 1) * SQ],
                                start=(ft == 0),
                                stop=(ft == NFT - 1),
                            )
                        nc.scalar.activation(
                            qt_sb[b][:, at * SQ : (at + 1) * SQ], ps[:],
                            AF.Identity, bias=bq_sb[:, at : at + 1],
                        )

            # ============ phase B: dots (fp16) + softmax over batch ============
            with (
                tc.tile_pool(name="phB", bufs=1) as phB,
                tc.tile_pool(name="ktp", bufs=3) as ktp,
                tc.tile_pool(name="scr", bufs=3) as scr,
            ):
                E = [[phB.tile([128, SQ], BF16, tag=f"E{b}_{kt}", name=f"E{b}_{kt}")
                      for kt in range(NKT)] for b in range(B)]
                for c in range(NCORES):
                    ktc = ktp.tile([128, B * NAT * SQ], F16, tag="ktc",
                                   name=f"ktc_{c}")
                    keng = nc.sync if (c % 2 == 0) else nc.scalar
                    keng.dma_start(
                        ktc[:].rearrange("p (bl at k) -> p bl at k", bl=B, at=NAT),
                        kg[c].rearrange("bl p at k -> p bl at k"),
                    )
                    for b in range(B):
                        for ktl in range(2):
                            kt = c * 2 + ktl
                            ps = psAp.tile([128, SQ], F32, tag="pk", bufs=6)
                            boff = b * NAT * SQ
                            for at in range(NAT):
                                nc.tensor.matmul(
                                    ps[:],
                                    ktc[:, boff + at * SQ + ktl * 128 :
                                        boff + at * SQ + ktl * 128 + 128],
                                    qt_sb[b][:, at * SQ : (at + 1) * SQ],
                                    start=(at == 0),
                                    stop=(at == NAT - 1),
                                )
                            nc.scalar.activation(
                                E[b][kt][:], ps[:], AF.Exp, bias=negc[:],
                            )
                            if b == B - 1:
                                d01 = scr.tile([128, SQ], BF16, tag="d01")
                                nc.vector.tensor_add(d01[:], E[0][kt][:], E[1][kt][:])
                                d23 = scr.tile([128, SQ], BF16, tag="d23")
                                nc.vector.tensor_add(d23[:], E[2][kt][:], E[3][kt][:])
                                dd = scr.tile([128, SQ], F32, tag="dd")
                                nc.vector.tensor_add(dd[:], d01[:], d23[:])
                                rr = scr.tile([128, SQ], F32, tag="rr")
                                nc.vector.reciprocal_approx_fast(rr[:], dd[:])
                                rb = scr.tile([128, SQ], BF16, tag="rb")
                                nc.vector.tensor_copy(rb[:], rr[:])
                                for b2 in range(2):
                                    nc.vector.tensor_mul(
                                        W_[b2][kt][:], E[b2][kt][:], rb[:]
                                    )
                                for b2 in range(2, B):
                                    nc.gpsimd.tensor_mul(
                                        W_[b2][kt][:], E[b2][kt][:], rb[:]
                                    )

            psA.__exit__(None, None, None)
            # ============ phase C: attn @ V (bf16) ============
            # V tiles stream in per a-half (16 tiles of [128, B*AH]); each half
            # runs two quarter accumulation passes (8 PSUM banks each) over the
            # same SBUF-resident tiles.
            psC = tc.tile_pool(name="psC", bufs=1, space="PSUM")
            psCp = psC.__enter__()
            with (
                tc.tile_pool(name="phC", bufs=1) as phC,
                tc.tile_pool(name="vtp", bufs=18) as vtp,
            ):
                attT = [phC.tile([128, NAT * SQ], BF16, tag=f"attT{b}",
                                 name=f"attT{b}") for b in range(B)]
                for h in range(2):
                    vts = {}
                    for c in range(NCORES):
                        for ktl in range(2):
                            vt = vtp.tile([128, B * AH], BF16, tag="vt",
                                          name=f"vt_{h}_{c}_{ktl}")
                            veng = nc.sync if (c % 2 == 0) else nc.scalar
                            veng.dma_start(
                                vt[:].rearrange("p (b a) -> p b a", b=B),
                                vg[h][c, :, ktl, :, :],
                            )
                            vts[(c, ktl)] = vt
                    for ql in range(2):
                        qtr = h * 2 + ql
                        ps_ba = [[psCp.tile([128, SQ], F32, tag=f"av{b}_{ai}",
                                            bufs=1, name=f"av{qtr}_{b}_{ai}")
                                  for ai in range(2)] for b in range(B)]
                        for c in range(NCORES):
                            for ktl in range(2):
                                kt = c * 2 + ktl
                                vt = vts[(c, ktl)]
                                for b in range(B):
                                    for ai in range(2):
                                        nc.tensor.matmul(
                                            ps_ba[b][ai][:],
                                            vt[:, b * AH + ql * 256 + ai * 128 :
                                               b * AH + ql * 256 + ai * 128 + 128],
                                            W_[b][kt][:],
                                            start=(kt == 0),
                                            stop=(kt == NKT - 1),
                                        )
                        for b in range(B):
                            for ai in range(2):
                                at = qtr * 2 + ai
                                nc.scalar.activation(
                                    attT[b][:, at * SQ : (at + 1) * SQ],
                                    ps_ba[b][ai][:], AF.Copy,
                                )

                psC.__exit__(None, None, None)
                # ---- output projection ----
                psO = tc.tile_pool(name="psO", bufs=1, space="PSUM")
                psOp = psO.__enter__()
                with tc.tile_pool(name="oout", bufs=4) as oout:
                    for b in range(B):
                        for qt in range(2):
                            for fc in range(2):
                                ps = psOp.tile([128, 512], F32, tag="po", bufs=2)
                                for at in range(NAT):
                                    nc.tensor.matmul(
                                        ps[:],
                                        attT[b][:, at * SQ + qt * 128 : at * SQ + qt * 128 + 128],
                                        wp_sb[:, at * F + fc * 512 : at * F + fc * 512 + 512],
                                        start=(at == 0),
                                        stop=False,
                                    )
                                nc.tensor.matmul(
                                    ps[:], ones_b[:], bp_sb[:, fc * 512 : fc * 512 + 512],
                                    start=False, stop=True,
                                )
                                o_t = oout.tile([128, 512], BF16, tag="ot")
                                nc.scalar.activation(o_t[:], ps[:], AF.Copy)
                                nc.sync.dma_start(
                                    out_ext[b, qt * 128 : (qt + 1) * 128,
                                            fc * 512 : (fc + 1) * 512],
                                    o_t[:],
                                )
                psO.__exit__(None, None, None)

    nc.finalize()
    return nc


_NC_CACHE = None


def _get_nc():
    global _NC_CACHE
    if _NC_CACHE is None:
        _NC_CACHE = build()
    return _NC_CACHE


def kernel(x, Wq, bq, Wk, bk, Wv, bv, Wp, bp, _trace=False):
    x = np.asarray(x, dtype=np.float32)
    Wq = np.asarray(Wq, dtype=np.float32)
    Wk = np.asarray(Wk, dtype=np.float32)
    Wv = np.asarray(Wv, dtype=np.float32)
    Wp = np.asarray(Wp, dtype=np.float32)
    bq = np.asarray(bq, dtype=np.float32)
    bk = np.asarray(bk, dtype=np.float32)
    bv = np.asarray(bv, dtype=np.float32)
    bp = np.asarray(bp, dtype=np.float32)

    # p-major packing: [p, ft, *] so SBUF loads are one contiguous run per
    # partition.
    def pmaj_w(w):  # [A_out, F_in] -> [128, NFT*A_out] (w.T tiled over f)
        return np.ascontiguousarray(
            w.reshape(A, NFT, 128).transpose(2, 1, 0).reshape(128, NFT * A)
        ).astype(np.float16)

    wqt = pmaj_w(Wq)
    wkt = pmaj_w(Wk)
    wvt = pmaj_w(Wv)
    wpt = np.ascontiguousarray(
        Wp.reshape(F, NAT, 128).transpose(2, 1, 0).reshape(128, NAT * F)
    ).astype(ml_dtypes.bfloat16)
    bq_p = np.ascontiguousarray(bq.reshape(NAT, 128).T)
    bk_p = np.ascontiguousarray(bk.reshape(NAT, 128).T)
    bv_p = bv.reshape(1, A).astype(np.float16)
    bp_p = bp.reshape(1, F).astype(ml_dtypes.bfloat16)
    ones_h = np.ones((1, 128), np.float16)
    ones_b = np.ones((1, 128), ml_dtypes.bfloat16)

    in_maps = []
    for c in range(NCORES):
        xs = x[:, c * SQ : (c + 1) * SQ, :]  # [B, SQ, F]
        xt_c = np.ascontiguousarray(
            xs.reshape(B, SQ, NFT, 128).transpose(0, 3, 2, 1).reshape(B, 128, NFT * SQ)
        ).astype(np.float16)
        in_maps.append({
            "xt": xt_c, "wqt": wqt, "wkt": wkt, "wvt": wvt, "wpt": wpt,
            "bq": bq_p, "bk": bk_p, "bv": bv_p, "bp": bp_p,
            "ones_h": ones_h, "ones_b": ones_b,
        })

    nc = _get_nc()
    res = run_bass_kernel_spmd(
        nc, in_maps, core_ids=list(range(NCORES)), trace=_trace
    )
    out = np.concatenate(
        [res.results[c]["out"].astype(np.float32) for c in range(NCORES)], axis=1
    )
    if _trace:
        kernel.last_results = res
    return out


# revision 12
# speedup vs baseline: 1.2762x; 1.2762x over previous
"""Distributed attention kernel for Trainium2 (8 NeuronCores).

Computes, matching the reference:
    Q = x @ Wq.T + bq; K = x @ Wk.T + bk; V = x @ Wv.T + bv        [B,S,A]
    dots = Q @ K.T (per batch)                                      [B,S,S]
    attn = softmax(dots, axis=0)            # softmax over the BATCH dim
    out  = (attn @ V) @ Wp.T + bp                                   [B,S,F]

Sharding: sequence (rows of Q) is split across the 8 cores (256 rows each,
all 4 batches per core, since the batch-softmax couples batches). Each core
also computes K^T and V for its 256 sequence rows and all-gathers them.

Schedule: projections run K -> V -> Q so each gather's ~30-50us mesh latency
hides under the remaining projections.  Three collectives only (each op has
~15us fixed mesh overhead + transfer): K batches 0-1, K batches 2-3, V.
All DRAM<->SBUF transfers use p-major layouts (partition index outermost on
the host side) so each DMA is 128 descriptors of 2-16KB contiguous runs.

Matmul dtypes: x and Wq/Wk/Wv in fp16 (halves HBM load traffic vs fp32 while
keeping dots errors ~1e-2 absolute), dots in fp16, attn@V / output
projection in bf16.  All accumulation is fp32 in PSUM.  exp() is taken with
a constant -30 shift (softmax is shift-invariant; keeps e^x in fp32 range
for dots in [-82,90]).  Output is written bf16 and cast to f32 on host.
"""

import numpy as np
import ml_dtypes

import concourse.bass as bass
import concourse.tile as tile
from concourse import bacc, mybir
from concourse.bass_utils import run_bass_kernel_spmd

AF = mybir.ActivationFunctionType
F32 = mybir.dt.float32
F16 = mybir.dt.float16
BF16 = mybir.dt.bfloat16

B, S, F, A = 4, 2048, 1024, 1024
NCORES = 8
SQ = S // NCORES          # 256 q rows per core
NFT = F // 128            # 8 f-tiles
NAT = A // 128            # 8 a-tiles
NKT = S // 128            # 16 global k-tiles
AH = A // 2               # 512, a-halves for V
RG = [list(range(NCORES))]


def build():
    nc = bacc.Bacc("TRN2", target_bir_lowering=False, debug=False)

    # p-major layouts: leading dim is the SBUF partition index.
    xt_ext = nc.declare_dram_parameter("xt", [B, 128, NFT * SQ], F16, isOutput=False)
    wqt_ext = nc.declare_dram_parameter("wqt", [128, NFT * A], F16, isOutput=False)
    wkt_ext = nc.declare_dram_parameter("wkt", [128, NFT * A], F16, isOutput=False)
    wvt_ext = nc.declare_dram_parameter("wvt", [128, NFT * A], F16, isOutput=False)
    wpt_ext = nc.declare_dram_parameter("wpt", [128, NAT * F], BF16, isOutput=False)
    bq_ext = nc.declare_dram_parameter("bq", [128, NAT], F32, isOutput=False)
    bk_ext = nc.declare_dram_parameter("bk", [128, NAT], F32, isOutput=False)
    bv_ext = nc.declare_dram_parameter("bv", [1, A], F16, isOutput=False)
    bp_ext = nc.declare_dram_parameter("bp", [1, F], BF16, isOutput=False)
    ones_h_ext = nc.declare_dram_parameter("ones_h", [1, 128], F16, isOutput=False)
    ones_b_ext = nc.declare_dram_parameter("ones_b", [1, 128], BF16, isOutput=False)
    out_ext = nc.declare_dram_parameter("out", [B, SQ, F], BF16, isOutput=True)

    with tile.TileContext(nc) as tc:
        with (
            tc.tile_pool(name="dram", bufs=1, space="DRAM") as dram,
            tc.tile_pool(name="live", bufs=1) as live,
        ):
            # ------------- bounce + gather DRAM buffers (p-major) -------------
            # kb[b][p][at][k]; gathered kgX[c][bl][p][at][k]
            kbA = dram.tile([1, 128, NAT, SQ], F16, tag="kbA", name="kbA")
            kbB = dram.tile([3, 128, NAT, SQ], F16, tag="kbB", name="kbB")
            kg0 = dram.tile([NCORES, 1, 128, NAT, SQ], F16, addr_space="Shared",
                            tag="kg0", name="kg0")
            kg123 = dram.tile([NCORES, 3, 128, NAT, SQ], F16, addr_space="Shared",
                              tag="kg123", name="kg123")
            # vb[h][p][st][b][a-half]; gathered vg[h][c][p][st][b][a-half]
            vb = [dram.tile([128, 2, B, AH], BF16, tag=f"vb{h}", name=f"vb{h}")
                  for h in range(2)]
            vg = [dram.tile([NCORES, 128, 2, B, AH], BF16, addr_space="Shared",
                            tag=f"vg{h}", name=f"vg{h}") for h in range(2)]

            # ---------------- whole-kernel SBUF residents ----------------
            bq_sb = live.tile([128, NAT], F32)
            bk_sb = live.tile([128, NAT], F32)
            bv_sb = live.tile([1, A], F16)
            bp_sb = live.tile([1, F], BF16)
            ones_h = live.tile([1, 128], F16)
            ones_b = live.tile([1, 128], BF16)
            negc = live.tile([128, 1], F32)
            nc.gpsimd.memset(negc[:], -30.0)

            qt_sb = [live.tile([128, NAT * SQ], F16, tag=f"qt{b}", name=f"qt{b}")
                     for b in range(B)]
            W_ = [[live.tile([128, SQ], BF16, tag=f"W{b}_{kt}", name=f"W{b}_{kt}")
                   for kt in range(NKT)] for b in range(B)]
            wp_sb = live.tile([128, NAT * F], BF16, tag="wp", name="wp_sb")

            # ============ phase A: projections (K, V, Q) ============
            psA = tc.tile_pool(name="psA", bufs=1, space="PSUM")
            psAp = psA.__enter__()
            with (
                tc.tile_pool(name="phA", bufs=1) as phA,
                tc.tile_pool(name="kout", bufs=12) as kout,
                tc.tile_pool(name="vout", bufs=4) as vout,
            ):
                # wk on the gpsimd queue, xt + the rest on sync: both start
                # at t=0 so the K projection can begin early.
                wk_sb = phA.tile([128, NFT * A], F16, tag="wk", name="wk_sb")
                nc.gpsimd.dma_start(wk_sb[:], wkt_ext[:])
                nc.scalar.dma_start(bq_sb[:], bq_ext[:])
                nc.scalar.dma_start(bk_sb[:], bk_ext[:])
                nc.scalar.dma_start(bv_sb[:], bv_ext[:])
                nc.scalar.dma_start(bp_sb[:], bp_ext[:])
                nc.scalar.dma_start(ones_h[:], ones_h_ext[:])
                nc.scalar.dma_start(ones_b[:], ones_b_ext[:])
                xt_sb = []
                for b in range(B):
                    t = phA.tile([128, NFT * SQ], F16, tag=f"xt{b}", name=f"xt{b}")
                    eng = nc.sync if b < 2 else nc.scalar
                    eng.dma_start(t[:], xt_ext[b])
                    xt_sb.append(t)
                wv_sb = phA.tile([128, NFT * A], F16, tag="wv", name="wv_sb")
                nc.scalar.dma_start(wv_sb[:], wvt_ext[:])
                wq_sb = phA.tile([128, NFT * A], F16, tag="wq", name="wq_sb")
                nc.scalar.dma_start(wq_sb[:], wqt_ext[:])
                nc.scalar.dma_start(wp_sb[:], wpt_ext[:])

                # ---- K^T projection; gather b01 as soon as b0,b1 are done,
                # gather b23 after b2,b3.
                for b in range(B):
                    for at in range(NAT):
                        ps = psAp.tile([128, SQ], F32, tag="pk", bufs=6)
                        for ft in range(NFT):
                            nc.tensor.matmul(
                                ps[:],
                                wk_sb[:, (ft * NAT + at) * 128 : (ft * NAT + at) * 128 + 128],
                                xt_sb[b][:, ft * SQ : (ft + 1) * SQ],
                                start=(ft == 0),
                                stop=(ft == NFT - 1),
                            )
                        kt_t = kout.tile([128, SQ], F16, tag="ko")
                        nc.scalar.activation(
                            kt_t[:], ps[:], AF.Identity, bias=bk_sb[:, at : at + 1]
                        )
                        if b == 0:
                            nc.sync.dma_start(kbA[0, :, at, :], kt_t[:])
                        else:
                            nc.sync.dma_start(kbB[b - 1, :, at, :], kt_t[:])
                    if b == 0:
                        nc.gpsimd.collective_compute(
                            "AllGather", mybir.AluOpType.bypass, replica_groups=RG,
                            ins=[kbA[:].opt()], outs=[kg0[:].opt()],
                        )
                    if b == B - 1:
                        nc.gpsimd.collective_compute(
                            "AllGather", mybir.AluOpType.bypass, replica_groups=RG,
                            ins=[kbB[:].opt()], outs=[kg123[:].opt()],
                        )

                # ---- V projection (a-half outer); per-half all-gather so the
                # first half's gather starts at the V-proj midpoint.
                for h in range(2):
                    for b in range(B):
                        for st in range(2):
                            ps = psAp.tile([128, AH], F32, tag="pv", bufs=2)
                            for ft in range(NFT):
                                nc.tensor.matmul(
                                    ps[:],
                                    xt_sb[b][:, ft * SQ + st * 128 : ft * SQ + st * 128 + 128],
                                    wv_sb[:, ft * A + h * AH : ft * A + h * AH + AH],
                                    start=(ft == 0),
                                    stop=False,
                                )
                            nc.tensor.matmul(
                                ps[:], ones_h[:], bv_sb[:, h * AH : h * AH + AH],
                                start=False, stop=True,
                            )
                            v_t = vout.tile([128, AH], BF16, tag="vo")
                            nc.scalar.activation(v_t[:], ps[:], AF.Copy)
                            nc.scalar.dma_start(vb[h][:, st, b, :], v_t[:])
                    nc.gpsimd.collective_compute(
                        "AllGather", mybir.AluOpType.bypass, replica_groups=RG,
                        ins=[vb[h][:].opt()], outs=[vg[h][:].opt()],
                    )

                # ---- Q^T projection
                for b in range(B):
                    for at in range(NAT):
                        ps = psAp.tile([128, SQ], F32, tag="pk", bufs=6)
                        for ft in range(NFT):
                            nc.tensor.matmul(
                                ps[:],
                                wq_sb[:, (ft * NAT + at) * 128 : (ft * NAT + at) * 128 + 128],
                                xt_sb[b][:, ft * SQ : (ft + 1) * SQ],
                                start=(ft == 0),
                                stop=(ft == NFT - 1),
                            )
                        nc.scalar.activation(
                            qt_sb[b][:, at * SQ : (at + 1) * SQ], ps[:],
                            AF.Identity, bias=bq_sb[:, at : at + 1],
                        )

            # ============ phase B: dots (fp16) + softmax over batch ============
            with (
                tc.tile_pool(name="phB", bufs=1) as phB,
                tc.tile_pool(name="ktpA", bufs=3) as ktpA,
                tc.tile_pool(name="ktpB", bufs=3) as ktpB,
                tc.tile_pool(name="scr", bufs=3) as scr,
            ):
                E = [[phB.tile([128, SQ], BF16, tag=f"E{b}_{kt}", name=f"E{b}_{kt}")
                      for kt in range(NKT)] for b in range(B)]
                # pass 1: batch 0 dots (kg0 arrives first)
                for c in range(NCORES):
                    ktcA = ktpA.tile([128, NAT * SQ], F16, tag="ktcA",
                                     name=f"ktcA_{c}")
                    keng = nc.sync if (c % 2 == 0) else nc.scalar
                    keng.dma_start(
                        ktcA[:].rearrange("p (at k) -> p at k", at=NAT),
                        kg0[c, 0],
                    )
                    for ktl in range(2):
                        kt = c * 2 + ktl
                        ps = psAp.tile([128, SQ], F32, tag="pk", bufs=6)
                        for at in range(NAT):
                            nc.tensor.matmul(
                                ps[:],
                                ktcA[:, at * SQ + ktl * 128 : at * SQ + ktl * 128 + 128],
                                qt_sb[0][:, at * SQ : (at + 1) * SQ],
                                start=(at == 0),
                                stop=(at == NAT - 1),
                            )
                        nc.scalar.activation(
                            E[0][kt][:], ps[:], AF.Exp, bias=negc[:],
                        )
                # pass 2: batches 1-3 + softmax spread over the c loop
                for c in range(NCORES):
                    ktcB = ktpB.tile([128, 3 * NAT * SQ], F16, tag="ktcB",
                                     name=f"ktcB_{c}")
                    keng = nc.sync if (c % 2 == 0) else nc.scalar
                    keng.dma_start(
                        ktcB[:].rearrange("p (bl at k) -> p bl at k", bl=3, at=NAT),
                        kg123[c].rearrange("bl p at k -> p bl at k"),
                    )
                    for b in range(1, B):
                        for ktl in range(2):
                            kt = c * 2 + ktl
                            ps = psAp.tile([128, SQ], F32, tag="pk", bufs=6)
                            boff = (b - 1) * NAT * SQ
                            for at in range(NAT):
                                nc.tensor.matmul(
                                    ps[:],
                                    ktcB[:, boff + at * SQ + ktl * 128 :
                                        boff + at * SQ + ktl * 128 + 128],
                                    qt_sb[b][:, at * SQ : (at + 1) * SQ],
                                    start=(at == 0),
                                    stop=(at == NAT - 1),
                                )
                            nc.scalar.activation(
                                E[b][kt][:], ps[:], AF.Exp, bias=negc[:],
                            )
                            if b == B - 1:
                                d01 = scr.tile([128, SQ], BF16, tag="d01")
                                nc.vector.tensor_add(d01[:], E[0][kt][:], E[1][kt][:])
                                d23 = scr.tile([128, SQ], BF16, tag="d23")
                                nc.vector.tensor_add(d23[:], E[2][kt][:], E[3][kt][:])
                                dd = scr.tile([128, SQ], F32, tag="dd")
                                nc.vector.tensor_add(dd[:], d01[:], d23[:])
                                rr = scr.tile([128, SQ], F32, tag="rr")
                                nc.vector.reciprocal_approx_fast(rr[:], dd[:])
                                rb = scr.tile([128, SQ], BF16, tag="rb")
                                nc.vector.tensor_copy(rb[:], rr[:])
                                for b2 in range(2):
                                    nc.vector.tensor_mul(
                                        W_[b2][kt][:], E[b2][kt][:], rb[:]
                                    )
                                for b2 in range(2, B):
                                    nc.gpsimd.tensor_mul(
                                        W_[b2][kt][:], E[b2][kt][:], rb[:]
                                    )

            psA.__exit__(None, None, None)
            # ============ phase C: attn @ V (bf16) ============
            # V tiles stream in per a-half (16 tiles of [128, B*AH]); each half
            # runs two quarter accumulation passes (8 PSUM banks each) over the
            # same SBUF-resident tiles.
            psC = tc.tile_pool(name="psC", bufs=1, space="PSUM")
            psCp = psC.__enter__()
            with (
                tc.tile_pool(name="phC", bufs=1) as phC,
                tc.tile_pool(name="vtp", bufs=18) as vtp,
            ):
                attT = [phC.tile([128, NAT * SQ], BF16, tag=f"attT{b}",
                                 name=f"attT{b}") for b in range(B)]
                for h in range(2):
                    vts = {}
                    for c in range(NCORES):
                        for ktl in range(2):
                            vt = vtp.tile([128, B * AH], BF16, tag="vt",
                                          name=f"vt_{h}_{c}_{ktl}")
                            veng = nc.sync if (c % 2 == 0) else nc.scalar
                            veng.dma_start(
                                vt[:].rearrange("p (b a) -> p b a", b=B),
                                vg[h][c, :, ktl, :, :],
                            )
                            vts[(c, ktl)] = vt
                    for ql in range(2):
                        qtr = h * 2 + ql
                        ps_ba = [[psCp.tile([128, SQ], F32, tag=f"av{b}_{ai}",
                                            bufs=1, name=f"av{qtr}_{b}_{ai}")
                                  for ai in range(2)] for b in range(B)]
                        for c in range(NCORES):
                            for ktl in range(2):
                                kt = c * 2 + ktl
                                vt = vts[(c, ktl)]
                                for b in range(B):
                                    for ai in range(2):
                                        nc.tensor.matmul(
                                            ps_ba[b][ai][:],
                                            vt[:, b * AH + ql * 256 + ai * 128 :
                                               b * AH + ql * 256 + ai * 128 + 128],
                                            W_[b][kt][:],
                                            start=(kt == 0),
                                            stop=(kt == NKT - 1),
                                        )
                        for b in range(B):
                            for ai in range(2):
                                at = qtr * 2 + ai
                                nc.scalar.activation(
                                    attT[b][:, at * SQ : (at + 1) * SQ],
                                    ps_ba[b][ai][:], AF.Copy,
                                )

                psC.__exit__(None, None, None)
                # ---- output projection ----
                psO = tc.tile_pool(name="psO", bufs=1, space="PSUM")
                psOp = psO.__enter__()
                with tc.tile_pool(name="oout", bufs=4) as oout:
                    for b in range(B):
                        for qt in range(2):
                            for fc in range(2):
                                ps = psOp.tile([128, 512], F32, tag="po", bufs=2)
                                for at in range(NAT):
                                    nc.tensor.matmul(
                                        ps[:],
                                        attT[b][:, at * SQ + qt * 128 : at * SQ + qt * 128 + 128],
                                        wp_sb[:, at * F + fc * 512 : at * F + fc * 512 + 512],
                                        start=(at == 0),
                                        stop=False,
                                    )
                                nc.tensor.matmul(
                                    ps[:], ones_b[:], bp_sb[:, fc * 512 : fc * 512 + 512],
                                    start=False, stop=True,
                                )
                                o_t = oout.tile([128, 512], BF16, tag="ot")
                                nc.scalar.activation(o_t[:], ps[:], AF.Copy)
                                nc.sync.dma_start(
                                    out_ext[b, qt * 128 : (qt + 1) * 128,
                                            fc * 512 : (fc + 1) * 512],
                                    o_t[:],
                                )
                psO.__exit__(None, None, None)

    nc.finalize()
    return nc


_NC_CACHE = None


def _get_nc():
    global _NC_CACHE
    if _NC_CACHE is None:
        _NC_CACHE = build()
    return _NC_CACHE


def kernel(x, Wq, bq, Wk, bk, Wv, bv, Wp, bp, _trace=False):
    x = np.asarray(x, dtype=np.float32)
    Wq = np.asarray(Wq, dtype=np.float32)
    Wk = np.asarray(Wk, dtype=np.float32)
    Wv = np.asarray(Wv, dtype=np.float32)
    Wp = np.asarray(Wp, dtype=np.float32)
    bq = np.asarray(bq, dtype=np.float32)
    bk = np.asarray(bk, dtype=np.float32)
    bv = np.asarray(bv, dtype=np.float32)
    bp = np.asarray(bp, dtype=np.float32)

    # p-major packing: [p, ft, *] so SBUF loads are one contiguous run per
    # partition.
    def pmaj_w(w):  # [A_out, F_in] -> [128, NFT*A_out] (w.T tiled over f)
        return np.ascontiguousarray(
            w.reshape(A, NFT, 128).transpose(2, 1, 0).reshape(128, NFT * A)
        ).astype(np.float16)

    wqt = pmaj_w(Wq)
    wkt = pmaj_w(Wk)
    wvt = pmaj_w(Wv)
    wpt = np.ascontiguousarray(
        Wp.reshape(F, NAT, 128).transpose(2, 1, 0).reshape(128, NAT * F)
    ).astype(ml_dtypes.bfloat16)
    bq_p = np.ascontiguousarray(bq.reshape(NAT, 128).T)
    bk_p = np.ascontiguousarray(bk.reshape(NAT, 128).T)
    bv_p = bv.reshape(1, A).astype(np.float16)
    bp_p = bp.reshape(1, F).astype(ml_dtypes.bfloat16)
    ones_h = np.ones((1, 128), np.float16)
    ones_b = np.ones((1, 128), ml_dtypes.bfloat16)

    in_maps = []
    for c in range(NCORES):
        xs = x[:, c * SQ : (c + 1) * SQ, :]  # [B, SQ, F]
        xt_c = np.ascontiguousarray(
            xs.reshape(B, SQ, NFT, 128).transpose(0, 3, 2, 1).reshape(B, 128, NFT * SQ)
        ).astype(np.float16)
        in_maps.append({
            "xt": xt_c, "wqt": wqt, "wkt": wkt, "wvt": wvt, "wpt": wpt,
            "bq": bq_p, "bk": bk_p, "bv": bv_p, "bp": bp_p,
            "ones_h": ones_h, "ones_b": ones_b,
        })

    nc = _get_nc()
    res = run_bass_kernel_spmd(
        nc, in_maps, core_ids=list(range(NCORES)), trace=_trace
    )
    out = np.concatenate(
        [res.results[c]["out"].astype(np.float32) for c in range(NCORES)], axis=1
    )
    if _trace:
        kernel.last_results = res
    return out
